# revision 11
# baseline (speedup 1.0000x reference)
"""Trainium2 Bass kernel for nn_EquiConv (e3nn-style FullyConnectedTensorProduct
+ gate + radial-MLP elementwise conv), data-parallel over edges on 8 cores.

v4 architecture (per core, 256-edge supertiles of two 128-edge subtiles):
  - ss/vv paths: DVE/GPSIMD build per-edge outer-product krons edge-major
    (per subtile), one DMA-transpose per subtile flips all 40 k-chunks into a
    shared k-major SBUF supertile, PE runs 40 chunk matmuls at 256-wide.
  - sv/vs paths: factorized. PE contracts the 64-wide scalar side first
    (weights stationary, x1sT/x2sT moving, per subtile into PSUM), ACT evicts
    to a shared bf16 supertile, DVE/GPSIMD multiply by host-replicated
    x2v/x1v "stack" operands (one instr per path-i), PE selector-matmuls
    reduce over v and scatter slice rows into the vec PSUM feature-major.
  - MLP (3 matmuls + Silu on ACT) at 256 wide; gate + elementwise conv fused
    feature-major; bf16 outputs, host reassembles/transposes.
"""

import sys

sys.path.insert(0, "/opt/trn_rl_repo")

import numpy as np
import ml_dtypes

import concourse.bass as bass
import concourse.bacc as bacc
import concourse.mybir as mybir
import concourse.tile as tile
from concourse.bass_utils import run_bass_kernel_spmd

BF16 = ml_dtypes.bfloat16

E = 20000
S = 64
V = 32
FC_IN = 128
HID = 64
INV_SQRT3 = 0.5773502691896258

NCORES = 8
EC = E // NCORES  # 2500 edges per core
ET = 128  # edges per subtile
NT = (EC + ET - 1) // ET  # 20 subtiles
EPAD = NT * ET  # 2560
NSUB = 2
EW = NSUB * ET  # 256 edges per supertile
NSUP = EPAD // EW  # 10

A_SC = float(1.0 / np.sqrt(np.float32(S * S + V * V)))
A_VEC = float(1.0 / np.sqrt(np.float32(2 * S * V)))

f32 = mybir.dt.float32
bf16 = mybir.dt.bfloat16

N_SS = (S * S) // 128  # 32 ss chunks
N_VV = (V * V) // 128  # 8 vv chunks (i-summed)
NCH = N_SS + N_VV      # 40 chunks -> 96-wide out (sc|g)
NSL = 8                # (dw,v) slices per sv/vs step-1 (8 x 128 rows)

# packed-constant column offsets (bf16 [128, WCONST])
OFF_WSSVV = 0
OFF_WSV = OFF_WSSVV + NCH * (S + V)
OFF_WVS = OFF_WSV + NSL * 4 * V
OFF_SEL = OFF_WVS + NSL * 4 * V
OFF_FC1 = OFF_SEL + NSL * V
OFF_FC2 = OFF_FC1 + HID
OFF_FC3 = OFF_FC2 + HID
OFF_SEL3 = OFF_FC3 + S + V
WCONST = OFF_SEL3 + 3 * V


def _prep_weights(w_ss_s, w_vv_s, w_ss_g, w_vv_g, w_sv_v, w_vs_v,
                  fc_w1, fc_b1, fc_w2, fc_b2, fc_w3, fc_b3):
    """Host-side rearrangement of the shared weights."""
    wss = np.concatenate([w_ss_s, w_ss_g], axis=2) * A_SC  # [64,64,96]
    wvv = np.concatenate([w_vv_s, w_vv_g], axis=2) * (A_SC * INV_SQRT3)  # [32,32,96]
    w_ssvv = np.concatenate(
        [wss.reshape(S * S, S + V), wvv.reshape(V * V, S + V)], axis=0
    )  # [5120, 96];  k = u*64+v (ss) ++ 4096 + u*32+v (vv)
    w_ssvv = (
        w_ssvv.reshape(NCH, 128, S + V).transpose(1, 0, 2)
        .reshape(128, NCH * (S + V))
    )

    # sv step1 stationary: [u, (s,dw,v)] = w_sv_v[u, v, s*4+dw] * A_VEC
    wsv_mat = (w_sv_v * A_VEC).transpose(0, 2, 1).reshape(S, NSL * 4 * V)
    # vs step1 stationary: [vs, (s,dw,uv)] = w_vs_v[uv, vs, s*4+dw] * A_VEC
    wvs_mat = (w_vs_v * A_VEC).transpose(1, 2, 0).reshape(S, NSL * 4 * V)

    # selector for the v-reduce: sel[p=(dw,v), s, w'] = 1 iff w' == s*4 + p//32
    sel = np.zeros((128, NSL, V), dtype=np.float32)
    for p in range(128):
        dw = p // 32
        for s in range(NSL):
            sel[p, s, s * 4 + dw] = 1.0

    sel3 = np.zeros((V, 3 * V), dtype=np.float32)  # replicate [32] -> [(i,w)=96]
    for i in range(3):
        for w in range(V):
            sel3[w, i * V + w] = 1.0

    wpack = np.zeros((128, WCONST), BF16)
    wpack[:, OFF_WSSVV:OFF_WSSVV + NCH * (S + V)] = w_ssvv.astype(BF16)
    wpack[0:S, OFF_WSV:OFF_WSV + NSL * 4 * V] = wsv_mat.astype(BF16)
    wpack[0:S, OFF_WVS:OFF_WVS + NSL * 4 * V] = wvs_mat.astype(BF16)
    wpack[:, OFF_SEL:OFF_SEL + NSL * V] = sel.reshape(128, NSL * V).astype(BF16)
    wpack[0:FC_IN, OFF_FC1:OFF_FC1 + HID] = fc_w1.astype(BF16)
    wpack[0:HID, OFF_FC2:OFF_FC2 + HID] = fc_w2.astype(BF16)
    wpack[0:HID, OFF_FC3:OFF_FC3 + S + V] = fc_w3.astype(BF16)
    wpack[0:V, OFF_SEL3:OFF_SEL3 + 3 * V] = sel3.astype(BF16)
    bpack = np.zeros((S + V, 3), np.float32)
    bpack[0:HID, 0] = fc_b1
    bpack[0:HID, 1] = fc_b2
    bpack[:, 2] = fc_b3
    return {"wpack": wpack, "bpack": bpack}


def _build_program():
    nc = bacc.Bacc("TRN2", target_bir_lowering=False, debug=False)

    d_fea = nc.dram_tensor("fea", [EPAD, 320], bf16, kind="ExternalInput").ap()
    d_feaT = nc.dram_tensor("feaT", [NSUP, S, 2, EW], bf16, kind="ExternalInput").ap()
    d_stk = nc.dram_tensor("stk", [NSUP, 128, 6, EW], bf16, kind="ExternalInput").ap()
    d_fwT = nc.dram_tensor("fwT", [FC_IN, EPAD], bf16, kind="ExternalInput").ap()
    d_wpack = nc.dram_tensor("wpack", [128, WCONST], bf16, kind="ExternalInput").ap()
    d_bpack = nc.dram_tensor("bpack", [S + V, 3], f32, kind="ExternalInput").ap()

    d_osc = nc.dram_tensor("out_sc", [S, EPAD], bf16, kind="ExternalOutput").ap()
    d_ovec = nc.dram_tensor("out_vec", [3 * V, EPAD], bf16, kind="ExternalOutput").ap()

    SiLU = mybir.ActivationFunctionType.Silu
    Sigm = mybir.ActivationFunctionType.Sigmoid
    Copy = mybir.ActivationFunctionType.Copy
    Ident = mybir.ActivationFunctionType.Identity
    mul_op = mybir.AluOpType.mult
    add_op = mybir.AluOpType.add

    with tile.TileContext(nc) as tc:
        with (
            tc.tile_pool(name="consts", bufs=1) as consts,
            tc.tile_pool(name="io", bufs=4) as io,
            tc.tile_pool(name="kron", bufs=2) as kronp,
            tc.tile_pool(name="ktr", bufs=2) as ktrp,
            tc.tile_pool(name="tsb", bufs=2) as tsbp,
            tc.tile_pool(name="mm", bufs=2) as mmp,
            tc.tile_pool(name="post", bufs=5) as postp,
            tc.tile_pool(name="pT", bufs=2, space=bass.MemorySpace.PSUM) as pT,
            tc.tile_pool(name="pacc", bufs=2, space=bass.MemorySpace.PSUM) as pacc,
            tc.tile_pool(name="pmlp", bufs=2, space=bass.MemorySpace.PSUM) as pmlp,
        ):
            # ---- constants (resident, one packed bf16 DMA + one f32 DMA) ----
            wpack = consts.tile([128, WCONST], bf16, name="wpack")
            nc.sync.dma_start(wpack[:], d_wpack)
            wssvv = wpack[:, OFF_WSSVV:OFF_WSSVV + NCH * (S + V)]
            wsv = wpack[0:S, OFF_WSV:OFF_WSV + NSL * 4 * V]
            wvs = wpack[0:S, OFF_WVS:OFF_WVS + NSL * 4 * V]
            sel = wpack[:, OFF_SEL:OFF_SEL + NSL * V]
            wfc1 = wpack[0:FC_IN, OFF_FC1:OFF_FC1 + HID]
            wfc2 = wpack[0:HID, OFF_FC2:OFF_FC2 + HID]
            wfc3 = wpack[0:HID, OFF_FC3:OFF_FC3 + S + V]
            sel3 = wpack[0:V, OFF_SEL3:OFF_SEL3 + 3 * V]
            bpack = consts.tile([S + V, 3], f32, name="bpack")
            nc.sync.dma_start(bpack[:], d_bpack)
            bfc1 = bpack[0:HID, 0:1]
            bfc2 = bpack[0:HID, 1:2]
            bfc3 = bpack[0:S + V, 2:3]

            state = {}

            def load_fwT(sp):
                fwT = io.tile([FC_IN, EW], bf16, tag="fwT", name=f"fwT_{sp}")
                nc.sync.dma_start(fwT[:], d_fwT[:, sp * EW:sp * EW + EW])
                state.setdefault(sp, {})["fwT"] = fwT

            def loads(sp):
                st = state.setdefault(sp, {})
                g0 = sp * EW
                feaT = io.tile([S, 2, EW], bf16, tag="feaT", name=f"feaT_{sp}")
                nc.sync.dma_start(feaT[:], d_feaT[sp])
                stk = io.tile([128, 6, EW], bf16, tag="stk", name=f"stk_{sp}")
                nc.sync.dma_start(stk[:], d_stk[sp])
                st["feaT"], st["stk"] = feaT, stk
                st["fea"] = []
                for s in range(NSUB):
                    e0 = g0 + s * ET
                    fea = io.tile([ET, 320], bf16, tag=f"fea_{s}", name=f"fea_{sp}_{s}")
                    nc.sync.dma_start(fea[:], d_fea[e0:e0 + ET, :])
                    st["fea"].append(fea)

            def mlp1(sp):
                st = state[sp]
                h1p = pmlp.tile([S + V, EW], f32, tag="mlp", name=f"h1p_{sp}")
                nc.tensor.matmul(h1p[0:HID, :], wfc1, st["fwT"][:], start=True, stop=True)
                h1b = postp.tile([HID, EW], bf16, tag="h1b", name=f"h1b_{sp}")
                nc.scalar.activation(h1b[:], h1p[0:HID, :], Ident, bias=bfc1)
                h1g = postp.tile([HID, EW], bf16, tag="h1g", name=f"h1g_{sp}")
                nc.scalar.activation(h1g[:], h1p[0:HID, :], Sigm, bias=bfc1)
                h1 = postp.tile([HID, EW], bf16, tag="h1", name=f"h1_{sp}")
                nc.gpsimd.tensor_tensor(h1[:], h1b[:], h1g[:], mul_op)
                st["h1"] = h1

            def mlp2(sp):
                st = state[sp]
                h2p = pmlp.tile([S + V, EW], f32, tag="mlp", name=f"h2p_{sp}")
                nc.tensor.matmul(h2p[0:HID, :], wfc2, st["h1"][:], start=True, stop=True)
                h2b = postp.tile([HID, EW], bf16, tag="h2b", name=f"h2b_{sp}")
                nc.scalar.activation(h2b[:], h2p[0:HID, :], Ident, bias=bfc2)
                h2g = postp.tile([HID, EW], bf16, tag="h2g", name=f"h2g_{sp}")
                nc.scalar.activation(h2g[:], h2p[0:HID, :], Sigm, bias=bfc2)
                h2 = postp.tile([HID, EW], bf16, tag="h2", name=f"h2_{sp}")
                nc.gpsimd.tensor_tensor(h2[:], h2b[:], h2g[:], mul_op)
                st["h2"] = h2

            def mlp3(sp):
                st = state[sp]
                wp = pmlp.tile([S + V, EW], f32, tag="mlp", name=f"wp_{sp}")
                nc.tensor.matmul(wp[:], wfc3, st["h2"][:], start=True, stop=True)
                wgt_sc = postp.tile([S, EW], bf16, tag="wgt_sc", name=f"wgt_sc_{sp}")
                nc.scalar.activation(wgt_sc[:], wp[0:S, :], Ident, bias=bfc3[0:S, :])
                wgt_v = postp.tile([V, EW], bf16, tag="wgt_v", name=f"wgt_v_{sp}")
                nc.scalar.activation(wgt_v[:], wp[S:S + V, :], Ident, bias=bfc3[S:S + V, :])
                st["wgt_sc"], st["wgt_v"] = wgt_sc, wgt_v

            def prebuilds(sp):
                st = state[sp]
                st["pre"] = []
                for s in range(NSUB):
                    fea = st["fea"][s]
                    x1s2 = io.tile([ET, 2 * S], bf16, tag=f"x1s2_{s}", name=f"x1s2_{sp}_{s}")
                    nc.scalar.activation(
                        x1s2[:].rearrange("e (u p) -> e u p", p=2),
                        fea[:, 0:S].unsqueeze(2).broadcast_to([ET, S, 2]), Copy)
                    x2vg = io.tile([ET, 3 * V], bf16, tag=f"x2vg_{s}", name=f"x2vg_{sp}_{s}")
                    nc.scalar.activation(
                        x2vg[:].rearrange("e (i u) -> e i u", u=V),
                        fea[:, 160 + S:320].rearrange("e (u i) -> e i u", i=3), Copy)
                    x1vg2 = io.tile([ET, 6 * V], bf16, tag=f"x1vg2_{s}", name=f"x1vg2_{sp}_{s}")
                    nc.scalar.activation(
                        x1vg2[:].rearrange("e (i u p) -> e i u p", u=V, p=2),
                        fea[:, S:160].rearrange("e (u i) -> e i u", i=3)
                            .unsqueeze(3).broadcast_to([ET, 3, V, 2]), Copy)
                    st["pre"].append((x1s2, x2vg, x1vg2))

            def krons(sp, s):
                st = state[sp]
                fea = st["fea"][s]
                x1s2, x2vg, x1vg2 = st["pre"][s]
                x2s = fea[:, 160:160 + S]
                if s == 0:
                    st["st_k"] = ktrp.tile([128, NCH, EW], bf16, tag="st_k", name=f"st_k_{sp}")
                kron = kronp.tile([ET, 5120], bf16, tag="kron", name=f"kron_{sp}_{s}")
                nc.vector.tensor_tensor(
                    kron[:, 0:S * S].rearrange("e (u vh p) -> e u vh p", vh=S // 2, p=2),
                    x1s2[:].rearrange("e (u p) -> e u p", p=2)
                        .unsqueeze(2).broadcast_to([ET, S, S // 2, 2]),
                    x2s.rearrange("e (vh p) -> e vh p", p=2)
                        .unsqueeze(1).broadcast_to([ET, S, S // 2, 2]),
                    mul_op)
                pv = [kronp.tile([ET, V * V], bf16, tag=f"pv{i}", name=f"pv{i}_{sp}_{s}") for i in range(2)]
                for i in range(3):
                    dst = (kron[:, S * S:S * S + V * V] if i == 2 else pv[i][:])
                    eng = nc.vector
                    eng.tensor_tensor(
                        dst.rearrange("e (u vh p) -> e u vh p", vh=V // 2, p=2),
                        x1vg2[:, i * 2 * V:(i + 1) * 2 * V]
                            .rearrange("e (u p) -> e u p", p=2)
                            .unsqueeze(2).broadcast_to([ET, V, V // 2, 2]),
                        x2vg[:, i * V:(i + 1) * V]
                            .rearrange("e (vh p) -> e vh p", p=2)
                            .unsqueeze(1).broadcast_to([ET, V, V // 2, 2]),
                        mul_op)
                kvv = kron[:, S * S:S * S + V * V]
                nc.vector.tensor_tensor(kvv, kvv, pv[0][:], add_op)
                nc.vector.tensor_tensor(kvv, kvv, pv[1][:], add_op)
                nc.sync.dma_start(st["st_k"][:, :, s * ET:(s + 1) * ET], kron[:],
                                  transpose=True)

            def step1(sp, s):
                st = state[sp]
                feaT = st["feaT"]
                if s == 0:
                    st["Tsv"] = tsbp.tile([128, NSL, EW], bf16, tag="Tsv", name=f"Tsv_sb_{sp}")
                    st["Tvs"] = tsbp.tile([128, NSL, EW], bf16, tag="Tvs", name=f"Tvs_sb_{sp}")
                Tsv, Tvs = st["Tsv"], st["Tvs"]
                Tsv_p = pT.tile([128, NSL * ET], f32, tag="T", name=f"Tsv_{sp}_{s}")
                for sl in range(NSL):
                    nc.tensor.matmul(
                        Tsv_p[:, sl * ET:(sl + 1) * ET],
                        wsv[:, sl * 128:(sl + 1) * 128],
                        feaT[:, 0, s * ET:(s + 1) * ET],
                        start=True, stop=True)
                nc.scalar.activation(Tsv[:, :, s * ET:(s + 1) * ET],
                                     Tsv_p[:].rearrange("p (sl e) -> p sl e", e=ET),
                                     Copy)
                Tvs_p = pT.tile([128, NSL * ET], f32, tag="T", name=f"Tvs_{sp}_{s}")
                for sl in range(NSL):
                    nc.tensor.matmul(
                        Tvs_p[:, sl * ET:(sl + 1) * ET],
                        wvs[:, sl * 128:(sl + 1) * 128],
                        feaT[:, 1, s * ET:(s + 1) * ET],
                        start=True, stop=True)
                nc.scalar.activation(Tvs[:, :, s * ET:(s + 1) * ET],
                                     Tvs_p[:].rearrange("p (sl e) -> p sl e", e=ET),
                                     Copy)

            def stage_B1(sp):
                st = state[sp]
                stk, Tsv, Tvs = st["stk"], st["Tsv"], st["Tvs"]
                acc = pacc.tile([S + V, 2 * EW], f32, tag="acc", name=f"acc_{sp}")
                st["acc"] = acc
                st["ms"] = []
                NGD = 4
                for i in range(3):
                    for jj, T in ((0, Tsv), (1, Tvs)):
                        j = 2 * i + jj
                        m = mmp.tile([128, NSL, EW], bf16, tag=f"m{j}", name=f"m{j}_{sp}")
                        nc.gpsimd.tensor_tensor(
                            m[:, NSL - NGD:NSL, :], T[:, NSL - NGD:NSL, :],
                            stk[:, j, :].unsqueeze(1).broadcast_to([128, NGD, EW]),
                            mul_op)
                        nc.vector.tensor_tensor(
                            m[:, 0:NSL - NGD, :], T[:, 0:NSL - NGD, :],
                            stk[:, j, :].unsqueeze(1).broadcast_to([128, NSL - NGD, EW]),
                            mul_op)
                        st["ms"].append(m)

            def stage_B2(sp):
                st = state[sp]
                acc_ss = st["acc"][:, 0:EW]
                st_k = st["st_k"]
                for c in range(NCH):
                    nc.tensor.matmul(
                        acc_ss,
                        wssvv[:, c * (S + V):(c + 1) * (S + V)],
                        st_k[:, c, :],
                        start=(c == 0), stop=(c == NCH - 1))

            def stage_B3(sp):
                st = state[sp]
                acc_v = st["acc"][:, EW:2 * EW]
                for i in range(3):
                    for jj in range(2):
                        m = st["ms"][2 * i + jj]
                        for sl in range(NSL):
                            nc.tensor.matmul(
                                acc_v[i * V:(i + 1) * V, :],
                                sel[:, sl * V:(sl + 1) * V],
                                m[:, sl, :],
                                start=(jj == 0 and sl == 0),
                                stop=(jj == 1 and sl == NSL - 1))

            def stage_C(sp):
                st = state.pop(sp)
                g0 = sp * EW
                acc = st["acc"]
                acc_v = acc[:, EW:2 * EW]
                wgt_sc, wgt_v = st["wgt_sc"], st["wgt_v"]
                sg_sc = postp.tile([S, EW], bf16, tag="sg_sc", name=f"sg_sc_{sp}")
                nc.scalar.activation(sg_sc[:], acc[0:S, 0:EW], Sigm)
                sg_g = postp.tile([V, EW], bf16, tag="sg_g", name=f"sg_g_{sp}")
                nc.scalar.activation(sg_g[:], acc[S:S + V, 0:EW], Sigm)

                sgw = postp.tile([S, EW], bf16, tag="sgw", name=f"sgw_{sp}")
                nc.vector.tensor_tensor(sgw[:], sg_sc[:], wgt_sc[:], mul_op)
                gwv = postp.tile([V, EW], bf16, tag="gwv", name=f"gwv_{sp}")
                nc.vector.tensor_tensor(gwv[:], sg_g[:], wgt_v[:], mul_op)
                gwrep_p = pmlp.tile([3 * V, EW], f32, tag="mlp", name=f"gwrep_{sp}")
                nc.tensor.matmul(gwrep_p[:], sel3, gwv[:], start=True, stop=True)
                gwrep = postp.tile([3 * V, EW], bf16, tag="gwrep", name=f"gwrep_sb_{sp}")
                nc.scalar.activation(gwrep[:], gwrep_p[:], Copy)

                accv_sb = postp.tile([3 * V, EW], bf16, tag="accv_sb", name=f"accv_sb_{sp}")
                nc.scalar.activation(accv_sb[:], acc_v[0:3 * V, :], Copy)
                accs_sb = postp.tile([S, EW], bf16, tag="accs_sb", name=f"accs_sb_{sp}")
                nc.scalar.activation(accs_sb[:], acc[0:S, 0:EW], Copy)
                osc = postp.tile([S, EW], bf16, tag="osc", name=f"osc_{sp}")
                nc.gpsimd.tensor_tensor(osc[:], accs_sb[:], sgw[:], mul_op)
                ovec = postp.tile([3 * V, EW], bf16, tag="ovec", name=f"ovec_{sp}")
                nc.vector.tensor_tensor(ovec[:], accv_sb[:], gwrep[:], mul_op)

                nc.sync.dma_start(d_osc[:, g0:g0 + EW], osc[:])
                nc.sync.dma_start(d_ovec[:, g0:g0 + EW], ovec[:])

            def due(base, r):
                # emit leg for sp where max(0, sp-base) == r
                if r == 0:
                    return [sp for sp in range(0, min(base + 1, NSUP))]
                sp = r + base
                return [sp] if sp < NSUP else []

            for sp in due(2, 0):
                load_fwT(sp)
            for r in range(NSUP + 2):
                if r < NSUP:
                    loads(r)
                for sp in due(2, r + 1):
                    load_fwT(sp)
                if 1 <= r <= NSUP:
                    stage_B1(r - 1)   # DVE/GPS mults first: ready at round start
                    stage_B2(r - 1)   # PE chunk matmuls: st_k ready
                if r < NSUP:
                    prebuilds(r)
                for sp in due(2, r):
                    mlp1(sp)
                if r < NSUP:
                    krons(r, 0)
                    step1(r, 0)
                    krons(r, 1)
                    step1(r, 1)
                if r >= 2:
                    stage_C(r - 2)
                if 1 <= r <= NSUP:
                    stage_B3(r - 1)   # PE reduces after this round's step1
                for sp in due(1, r):
                    mlp2(sp)
                for sp in due(0, r):
                    mlp3(sp)

    nc.compile()
    return nc


_CACHED = {}


def kernel(fea_in1, fea_in2, fea_weight,
           w_ss_s, w_vv_s, w_ss_g, w_vv_g, w_sv_v, w_vs_v,
           fc_w1, fc_b1, fc_w2, fc_b2, fc_w3, fc_b3, batch_edge):
    fea_in1 = np.asarray(fea_in1, dtype=np.float32)
    fea_in2 = np.asarray(fea_in2, dtype=np.float32)
    fea_weight = np.asarray(fea_weight, dtype=np.float32)

    wd = _prep_weights(np.asarray(w_ss_s, np.float32), np.asarray(w_vv_s, np.float32),
                       np.asarray(w_ss_g, np.float32), np.asarray(w_vv_g, np.float32),
                       np.asarray(w_sv_v, np.float32), np.asarray(w_vs_v, np.float32),
                       np.asarray(fc_w1, np.float32), np.asarray(fc_b1, np.float32),
                       np.asarray(fc_w2, np.float32), np.asarray(fc_b2, np.float32),
                       np.asarray(fc_w3, np.float32), np.asarray(fc_b3, np.float32))

    if "nc" not in _CACHED:
        _CACHED["nc"] = _build_program()
    nc = _CACHED["nc"]

    in_maps = []
    for c in range(NCORES):
        s0 = c * EC
        f1 = np.zeros((EPAD, 160), BF16)
        f1[:EC] = fea_in1[s0:s0 + EC].astype(BF16)
        f2 = np.zeros((EPAD, 160), BF16)
        f2[:EC] = fea_in2[s0:s0 + EC].astype(BF16)
        fea = np.concatenate([f1, f2], axis=1)  # [EPAD, 320]

        feaT = np.zeros((NSUP, S, 2, EW), BF16)
        feaT[:, :, 0, :] = f1[:, :S].reshape(NSUP, EW, S).transpose(0, 2, 1)
        feaT[:, :, 1, :] = f2[:, :S].reshape(NSUP, EW, S).transpose(0, 2, 1)

        # stacks: [NSUP, p=(dw,v), j, e]; j=2i -> x2v[e, v=p%32, i]; j=2i+1 -> x1v
        x1v = f1[:, S:].reshape(EPAD, V, 3)
        x2v = f2[:, S:].reshape(EPAD, V, 3)
        stk = np.empty((NSUP, 128, 6, EW), BF16)
        for i in range(3):
            s2 = x2v[:, :, i].T.reshape(1, V, NSUP, EW)
            s1 = x1v[:, :, i].T.reshape(1, V, NSUP, EW)
            stk[:, :, 2 * i, :] = np.broadcast_to(s2, (4, V, NSUP, EW)) \
                .reshape(128, NSUP, EW).transpose(1, 0, 2)
            stk[:, :, 2 * i + 1, :] = np.broadcast_to(s1, (4, V, NSUP, EW)) \
                .reshape(128, NSUP, EW).transpose(1, 0, 2)

        fwT = np.zeros((FC_IN, EPAD), BF16)
        fwT[:, :EC] = fea_weight[s0:s0 + EC].T.astype(BF16)
        m = {"fea": fea, "feaT": feaT, "stk": stk, "fwT": fwT}
        m.update(wd)
        in_maps.append(m)

    import os
    trace = bool(int(os.environ.get("KERNEL_TRACE", "0")))
    res = run_bass_kernel_spmd(nc, in_maps, core_ids=list(range(NCORES)), trace=trace)
    _CACHED["exec_time_ns"] = res.exec_time_ns

    out = np.empty((E, S + 3 * V), np.float32)
    # vec partition p = i*32+w  ->  output column 64 + 3*w + i
    vec_cols = np.empty(3 * V, np.int64)
    for i in range(3):
        for w in range(V):
            vec_cols[i * V + w] = S + 3 * w + i
    for c in range(NCORES):
        s0 = c * EC
        osc = np.asarray(res.results[c]["out_sc"], dtype=np.float32)[:, :EC]
        ovec = np.asarray(res.results[c]["out_vec"], dtype=np.float32)[:, :EC]
        out[s0:s0 + EC, :S] = osc.T
        out[s0:s0 + EC, vec_cols] = ovec.T
    return out


if __name__ == "__main__":
    rng = np.random.default_rng(0)
    ins = {
        "fea_in1": rng.standard_normal((E, 160)).astype(np.float32),
        "fea_in2": rng.standard_normal((E, 160)).astype(np.float32),
        "fea_weight": rng.standard_normal((E, FC_IN)).astype(np.float32),
        "w_ss_s": rng.standard_normal((S, S, S)).astype(np.float32),
        "w_vv_s": rng.standard_normal((V, V, S)).astype(np.float32),
        "w_ss_g": rng.standard_normal((S, S, V)).astype(np.float32),
        "w_vv_g": rng.standard_normal((V, V, V)).astype(np.float32),
        "w_sv_v": rng.standard_normal((S, V, V)).astype(np.float32),
        "w_vs_v": rng.standard_normal((V, S, V)).astype(np.float32),
        "fc_w1": rng.standard_normal((FC_IN, HID)).astype(np.float32),
        "fc_b1": np.zeros(HID, np.float32),
        "fc_w2": rng.standard_normal((HID, HID)).astype(np.float32),
        "fc_b2": np.zeros(HID, np.float32),
        "fc_w3": rng.standard_normal((HID, S + V)).astype(np.float32),
        "fc_b3": np.zeros(S + V, np.float32),
        "batch_edge": np.zeros(E, np.int32),
    }
    out = kernel(**ins)
    print("kernel out", out.shape, out.dtype, float(np.abs(out).mean()))


# revision 26
# speedup vs baseline: 1.0293x; 1.0293x over previous
"""Trainium2 Bass kernel for nn_EquiConv (e3nn-style FullyConnectedTensorProduct
+ gate + radial-MLP elementwise conv), data-parallel over edges on 8 cores.

v4 architecture (per core, 256-edge supertiles of two 128-edge subtiles):
  - ss/vv paths: DVE/GPSIMD build per-edge outer-product krons edge-major
    (per subtile), one DMA-transpose per subtile flips all 40 k-chunks into a
    shared k-major SBUF supertile, PE runs 40 chunk matmuls at 256-wide.
  - sv/vs paths: factorized. PE contracts the 64-wide scalar side first
    (weights stationary, x1sT/x2sT moving, per subtile into PSUM), ACT evicts
    to a shared bf16 supertile, DVE/GPSIMD multiply by host-replicated
    x2v/x1v "stack" operands (one instr per path-i), PE selector-matmuls
    reduce over v and scatter slice rows into the vec PSUM feature-major.
  - MLP (3 matmuls + Silu on ACT) at 256 wide; gate + elementwise conv fused
    feature-major; bf16 outputs, host reassembles/transposes.
"""

import sys

sys.path.insert(0, "/opt/trn_rl_repo")

import numpy as np
import ml_dtypes

import concourse.bass as bass
import concourse.bacc as bacc
import concourse.mybir as mybir
import concourse.tile as tile
from concourse.bass_utils import run_bass_kernel_spmd

BF16 = ml_dtypes.bfloat16

E = 20000
S = 64
V = 32
FC_IN = 128
HID = 64
INV_SQRT3 = 0.5773502691896258

NCORES = 8
EC = E // NCORES  # 2500 edges per core
ET = 128  # edges per subtile
NT = (EC + ET - 1) // ET  # 20 subtiles
EPAD = NT * ET  # 2560
NSUB = 2
EW = NSUB * ET  # 256 edges per supertile
NSUP = EPAD // EW  # 10

A_SC = float(1.0 / np.sqrt(np.float32(S * S + V * V)))
A_VEC = float(1.0 / np.sqrt(np.float32(2 * S * V)))

f32 = mybir.dt.float32
bf16 = mybir.dt.bfloat16

N_SS = (S * S) // 128  # 32 ss chunks
N_VV = (V * V) // 128  # 8 vv chunks (i-summed)
NCH = N_SS + N_VV      # 40 chunks -> 96-wide out (sc|g)
NSL = 8                # (dw,v) slices per sv/vs step-1 (8 x 128 rows)

# packed-constant column offsets (bf16 [128, WCONST])
OFF_WSSVV = 0
OFF_WSV = OFF_WSSVV + NCH * (S + V)
OFF_WVS = OFF_WSV + NSL * 4 * V
OFF_SEL = OFF_WVS + NSL * 4 * V
OFF_FC1 = OFF_SEL + NSL * V
OFF_FC2 = OFF_FC1 + HID
OFF_FC3 = OFF_FC2 + HID
OFF_SEL3 = OFF_FC3 + S + V
WCONST = OFF_SEL3 + 3 * V


def _prep_weights(w_ss_s, w_vv_s, w_ss_g, w_vv_g, w_sv_v, w_vs_v,
                  fc_w1, fc_b1, fc_w2, fc_b2, fc_w3, fc_b3):
    """Host-side rearrangement of the shared weights."""
    wss = np.concatenate([w_ss_s, w_ss_g], axis=2) * A_SC  # [64,64,96]
    wvv = np.concatenate([w_vv_s, w_vv_g], axis=2) * (A_SC * INV_SQRT3)  # [32,32,96]
    w_ssvv = np.concatenate(
        [wss.reshape(S * S, S + V), wvv.reshape(V * V, S + V)], axis=0
    )  # [5120, 96];  k = u*64+v (ss) ++ 4096 + u*32+v (vv)
    w_ssvv = (
        w_ssvv.reshape(NCH, 128, S + V).transpose(1, 0, 2)
        .reshape(128, NCH * (S + V))
    )

    # sv step1 stationary: [u, (s,dw,v)] = w_sv_v[u, v, s*4+dw] * A_VEC
    wsv_mat = (w_sv_v * A_VEC).transpose(0, 2, 1).reshape(S, NSL * 4 * V)
    # vs step1 stationary: [vs, (s,dw,uv)] = w_vs_v[uv, vs, s*4+dw] * A_VEC
    wvs_mat = (w_vs_v * A_VEC).transpose(1, 2, 0).reshape(S, NSL * 4 * V)

    # selector for the v-reduce: sel[p=(dw,v), s, w'] = 1 iff w' == s*4 + p//32
    sel = np.zeros((128, NSL, V), dtype=np.float32)
    for p in range(128):
        dw = p // 32
        for s in range(NSL):
            sel[p, s, s * 4 + dw] = 1.0

    sel3 = np.zeros((V, 3 * V), dtype=np.float32)  # replicate [32] -> [(i,w)=96]
    for i in range(3):
        for w in range(V):
            sel3[w, i * V + w] = 1.0

    wpack = np.zeros((128, WCONST), BF16)
    wpack[:, OFF_WSSVV:OFF_WSSVV + NCH * (S + V)] = w_ssvv.astype(BF16)
    wpack[0:S, OFF_WSV:OFF_WSV + NSL * 4 * V] = wsv_mat.astype(BF16)
    wpack[0:S, OFF_WVS:OFF_WVS + NSL * 4 * V] = wvs_mat.astype(BF16)
    wpack[:, OFF_SEL:OFF_SEL + NSL * V] = sel.reshape(128, NSL * V).astype(BF16)
    wpack[0:FC_IN, OFF_FC1:OFF_FC1 + HID] = fc_w1.astype(BF16)
    wpack[0:HID, OFF_FC2:OFF_FC2 + HID] = fc_w2.astype(BF16)
    wpack[0:HID, OFF_FC3:OFF_FC3 + S + V] = fc_w3.astype(BF16)
    wpack[0:V, OFF_SEL3:OFF_SEL3 + 3 * V] = sel3.astype(BF16)
    bpack = np.zeros((S + V, 3), np.float32)
    bpack[0:HID, 0] = fc_b1
    bpack[0:HID, 1] = fc_b2
    bpack[:, 2] = fc_b3
    return {"wpack": wpack, "bpack": bpack}


def _build_program():
    nc = bacc.Bacc("TRN2", target_bir_lowering=False, debug=False)

    d_fea = nc.dram_tensor("fea", [EPAD, 320], bf16, kind="ExternalInput").ap()
    d_feaT = nc.dram_tensor("feaT", [NSUP, S, 2, EW], bf16, kind="ExternalInput").ap()
    d_stk = nc.dram_tensor("stk", [NSUP, 128, 6, EW], bf16, kind="ExternalInput").ap()
    d_fwT = nc.dram_tensor("fwT", [FC_IN, EPAD], bf16, kind="ExternalInput").ap()
    d_wpack = nc.dram_tensor("wpack", [128, WCONST], bf16, kind="ExternalInput").ap()
    d_bpack = nc.dram_tensor("bpack", [S + V, 3], f32, kind="ExternalInput").ap()

    d_osc = nc.dram_tensor("out_sc", [S, EPAD], bf16, kind="ExternalOutput").ap()
    d_ovec = nc.dram_tensor("out_vec", [3 * V, EPAD], bf16, kind="ExternalOutput").ap()

    SiLU = mybir.ActivationFunctionType.Silu
    Sigm = mybir.ActivationFunctionType.Sigmoid
    Copy = mybir.ActivationFunctionType.Copy
    Ident = mybir.ActivationFunctionType.Identity
    mul_op = mybir.AluOpType.mult
    add_op = mybir.AluOpType.add

    with tile.TileContext(nc) as tc:
        with (
            tc.tile_pool(name="consts", bufs=1) as consts,
            tc.tile_pool(name="io", bufs=3) as io,
            tc.tile_pool(name="kron", bufs=2) as kronp,
            tc.tile_pool(name="ktr", bufs=2) as ktrp,
            tc.tile_pool(name="tsb", bufs=2) as tsbp,
            tc.tile_pool(name="mm", bufs=2) as mmp,
            tc.tile_pool(name="post", bufs=4) as postp,
            tc.tile_pool(name="pT", bufs=3, space=bass.MemorySpace.PSUM) as pT,
            tc.tile_pool(name="pacc", bufs=3, space=bass.MemorySpace.PSUM) as pacc,
            tc.tile_pool(name="pmlp", bufs=2, space=bass.MemorySpace.PSUM) as pmlp,
        ):
            # ---- constants (resident, one packed bf16 DMA + one f32 DMA) ----
            wpack = consts.tile([128, WCONST], bf16, name="wpack")
            nc.sync.dma_start(wpack[:], d_wpack)
            wssvv = wpack[:, OFF_WSSVV:OFF_WSSVV + NCH * (S + V)]
            wsv = wpack[0:S, OFF_WSV:OFF_WSV + NSL * 4 * V]
            wvs = wpack[0:S, OFF_WVS:OFF_WVS + NSL * 4 * V]
            sel = wpack[:, OFF_SEL:OFF_SEL + NSL * V]
            wfc1 = wpack[0:FC_IN, OFF_FC1:OFF_FC1 + HID]
            wfc2 = wpack[0:HID, OFF_FC2:OFF_FC2 + HID]
            wfc3 = wpack[0:HID, OFF_FC3:OFF_FC3 + S + V]
            sel3 = wpack[0:V, OFF_SEL3:OFF_SEL3 + 3 * V]
            bpack = consts.tile([S + V, 3], f32, name="bpack")
            nc.sync.dma_start(bpack[:], d_bpack)
            bfc1 = bpack[0:HID, 0:1]
            bfc2 = bpack[0:HID, 1:2]
            bfc3 = bpack[0:S + V, 2:3]

            state = {}

            def load_fwT(sp):
                fwT = io.tile([FC_IN, EW], bf16, tag="fwT", name=f"fwT_{sp}")
                nc.sync.dma_start(fwT[:], d_fwT[:, sp * EW:sp * EW + EW])
                state.setdefault(sp, {})["fwT"] = fwT

            def loads(sp):
                st = state.setdefault(sp, {})
                g0 = sp * EW
                feaT = io.tile([S, 2, EW], bf16, tag="feaT", name=f"feaT_{sp}")
                nc.sync.dma_start(feaT[:], d_feaT[sp])
                stk = io.tile([128, 6, EW], bf16, tag="stk", name=f"stk_{sp}")
                nc.sync.dma_start(stk[:], d_stk[sp])
                st["feaT"], st["stk"] = feaT, stk
                st["fea"] = []
                for s in range(NSUB):
                    e0 = g0 + s * ET
                    fea = io.tile([ET, 320], bf16, tag=f"fea_{s}", name=f"fea_{sp}_{s}")
                    nc.sync.dma_start(fea[:], d_fea[e0:e0 + ET, :])
                    st["fea"].append(fea)

            def mlp1(sp):
                st = state[sp]
                h1p = pmlp.tile([S + V, EW], f32, tag="mlp", name=f"h1p_{sp}")
                nc.tensor.matmul(h1p[0:HID, :], wfc1, st["fwT"][:], start=True, stop=True)
                h1b = postp.tile([HID, EW], bf16, tag="h1b", name=f"h1b_{sp}")
                nc.scalar.activation(h1b[:], h1p[0:HID, :], Ident, bias=bfc1)
                h1g = postp.tile([HID, EW], bf16, tag="h1g", name=f"h1g_{sp}")
                nc.scalar.activation(h1g[:], h1p[0:HID, :], Sigm, bias=bfc1)
                st["h1parts"] = (h1b, h1g)

            def mlp2(sp):
                st = state[sp]
                h2p = pmlp.tile([S + V, EW], f32, tag="mlp", name=f"h2p_{sp}")
                nc.tensor.matmul(h2p[0:HID, :], wfc2, st["h1"][:], start=True, stop=True)
                h2b = postp.tile([HID, EW], bf16, tag="h2b", name=f"h2b_{sp}")
                nc.scalar.activation(h2b[:], h2p[0:HID, :], Ident, bias=bfc2)
                h2g = postp.tile([HID, EW], bf16, tag="h2g", name=f"h2g_{sp}")
                nc.scalar.activation(h2g[:], h2p[0:HID, :], Sigm, bias=bfc2)
                st["h2parts"] = (h2b, h2g)

            def mlp1g(sp):
                st = state[sp]
                h1b, h1g = st["h1parts"]
                h1 = postp.tile([HID, EW], bf16, tag="h1", name=f"h1_{sp}")
                nc.gpsimd.tensor_tensor(h1[:], h1b[:], h1g[:], mul_op)
                st["h1"] = h1

            def mlp2g(sp):
                st = state[sp]
                h2b, h2g = st["h2parts"]
                h2 = postp.tile([HID, EW], bf16, tag="h2", name=f"h2_{sp}")
                nc.gpsimd.tensor_tensor(h2[:], h2b[:], h2g[:], mul_op)
                st["h2"] = h2

            def mlp3(sp):
                st = state[sp]
                wp = pmlp.tile([S + V, EW], f32, tag="mlp", name=f"wp_{sp}")
                nc.tensor.matmul(wp[:], wfc3, st["h2"][:], start=True, stop=True)
                wgt_sc = postp.tile([S, EW], bf16, tag="wgt_sc", name=f"wgt_sc_{sp}")
                nc.scalar.activation(wgt_sc[:], wp[0:S, :], Ident, bias=bfc3[0:S, :])
                wgt_v = postp.tile([V, EW], bf16, tag="wgt_v", name=f"wgt_v_{sp}")
                nc.scalar.activation(wgt_v[:], wp[S:S + V, :], Ident, bias=bfc3[S:S + V, :])
                st["wgt_sc"], st["wgt_v"] = wgt_sc, wgt_v

            def prebuilds(sp):
                st = state[sp]
                st["pre"] = []
                for s in range(NSUB):
                    fea = st["fea"][s]
                    x1s2 = io.tile([ET, 2 * S], bf16, tag=f"x1s2_{s}", name=f"x1s2_{sp}_{s}")
                    nc.scalar.activation(
                        x1s2[:].rearrange("e (u p) -> e u p", p=2),
                        fea[:, 0:S].unsqueeze(2).broadcast_to([ET, S, 2]), Copy)
                    x2vg = io.tile([ET, 3 * V], bf16, tag=f"x2vg_{s}", name=f"x2vg_{sp}_{s}")
                    nc.scalar.activation(
                        x2vg[:].rearrange("e (i u) -> e i u", u=V),
                        fea[:, 160 + S:320].rearrange("e (u i) -> e i u", i=3), Copy)
                    x1vg2 = io.tile([ET, 6 * V], bf16, tag=f"x1vg2_{s}", name=f"x1vg2_{sp}_{s}")
                    nc.scalar.activation(
                        x1vg2[:].rearrange("e (i u p) -> e i u p", u=V, p=2),
                        fea[:, S:160].rearrange("e (u i) -> e i u", i=3)
                            .unsqueeze(3).broadcast_to([ET, 3, V, 2]), Copy)
                    st["pre"].append((x1s2, x2vg, x1vg2))

            def krons(sp, s):
                st = state[sp]
                fea = st["fea"][s]
                x1s2, x2vg, x1vg2 = st["pre"][s]
                x2s = fea[:, 160:160 + S]
                if s == 0:
                    st["st_k"] = ktrp.tile([128, NCH, EW], bf16, tag="st_k", name=f"st_k_{sp}")
                kron = kronp.tile([ET, 5120], bf16, tag="kron", name=f"kron_{sp}_{s}")
                nc.vector.tensor_tensor(
                    kron[:, 0:S * S].rearrange("e (u vh p) -> e u vh p", vh=S // 2, p=2),
                    x1s2[:].rearrange("e (u p) -> e u p", p=2)
                        .unsqueeze(2).broadcast_to([ET, S, S // 2, 2]),
                    x2s.rearrange("e (vh p) -> e vh p", p=2)
                        .unsqueeze(1).broadcast_to([ET, S, S // 2, 2]),
                    mul_op)
                pv = [kronp.tile([ET, V * V], bf16, tag=f"pv{i}", name=f"pv{i}_{sp}_{s}") for i in range(2)]
                for i in range(3):
                    dst = (kron[:, S * S:S * S + V * V] if i == 2 else pv[i][:])
                    eng = nc.vector
                    eng.tensor_tensor(
                        dst.rearrange("e (u vh p) -> e u vh p", vh=V // 2, p=2),
                        x1vg2[:, i * 2 * V:(i + 1) * 2 * V]
                            .rearrange("e (u p) -> e u p", p=2)
                            .unsqueeze(2).broadcast_to([ET, V, V // 2, 2]),
                        x2vg[:, i * V:(i + 1) * V]
                            .rearrange("e (vh p) -> e vh p", p=2)
                            .unsqueeze(1).broadcast_to([ET, V, V // 2, 2]),
                        mul_op)
                kvv = kron[:, S * S:S * S + V * V]
                nc.vector.tensor_tensor(kvv, kvv, pv[0][:], add_op)
                nc.vector.tensor_tensor(kvv, kvv, pv[1][:], add_op)
                nc.sync.dma_start(st["st_k"][:, :, s * ET:(s + 1) * ET], kron[:],
                                  transpose=True)

            def step1(sp, s):
                st = state[sp]
                feaT = st["feaT"]
                if s == 0:
                    st["Tsv"] = tsbp.tile([128, NSL, EW], bf16, tag="Tsv", name=f"Tsv_sb_{sp}")
                    st["Tvs"] = tsbp.tile([128, NSL, EW], bf16, tag="Tvs", name=f"Tvs_sb_{sp}")
                Tsv, Tvs = st["Tsv"], st["Tvs"]
                NH = NSL // 2
                for path, (wmat, T) in enumerate(((wsv, Tsv), (wvs, Tvs))):
                    for h in range(2):
                        Tp = pT.tile([128, NH * ET], f32, tag="T", name=f"T_{sp}_{s}_{path}_{h}")
                        for sl in range(NH):
                            gsl = h * NH + sl
                            nc.tensor.matmul(
                                Tp[:, sl * ET:(sl + 1) * ET],
                                wmat[:, gsl * 128:(gsl + 1) * 128],
                                feaT[:, path, s * ET:(s + 1) * ET],
                                start=True, stop=True)
                        nc.scalar.activation(
                            T[:, h * NH:(h + 1) * NH, s * ET:(s + 1) * ET],
                            Tp[:].rearrange("p (sl e) -> p sl e", e=ET),
                            Copy)

            def stage_B1(sp, half):
                st = state[sp]
                stk, Tsv, Tvs = st["stk"], st["Tsv"], st["Tvs"]
                if half == 0:
                    st["ms"] = []
                NGD = 4
                for i in ((0, 1) if half == 0 else (2,)):
                    for jj, T in ((0, Tsv), (1, Tvs)):
                        j = 2 * i + jj
                        m = mmp.tile([128, NSL, EW], bf16, tag=f"m{j}", name=f"m{j}_{sp}")
                        nc.gpsimd.tensor_tensor(
                            m[:, NSL - NGD:NSL, :], T[:, NSL - NGD:NSL, :],
                            stk[:, j, :].unsqueeze(1).broadcast_to([128, NGD, EW]),
                            mul_op)
                        nc.vector.tensor_tensor(
                            m[:, 0:NSL - NGD, :], T[:, 0:NSL - NGD, :],
                            stk[:, j, :].unsqueeze(1).broadcast_to([128, NSL - NGD, EW]),
                            mul_op)
                        st["ms"].append(m)

            def stage_B2(sp, s):
                st = state[sp]
                if s == 0:
                    st["acc"] = pacc.tile([S + V, 2 * EW], f32, tag="acc", name=f"acc_{sp}")
                acc_ss = st["acc"][:, 0:EW]
                st_k = st["st_k"]
                for c in range(NCH):
                    nc.tensor.matmul(
                        acc_ss[:, s * ET:(s + 1) * ET],
                        wssvv[:, c * (S + V):(c + 1) * (S + V)],
                        st_k[:, c, s * ET:(s + 1) * ET],
                        start=(c == 0), stop=(c == NCH - 1))

            def stage_B3(sp):
                st = state[sp]
                acc_v = st["acc"][:, EW:2 * EW]
                for i in range(3):
                    for jj in range(2):
                        m = st["ms"][2 * i + jj]
                        for sl in range(NSL):
                            nc.tensor.matmul(
                                acc_v[i * V:(i + 1) * V, :],
                                sel[:, sl * V:(sl + 1) * V],
                                m[:, sl, :],
                                start=(jj == 0 and sl == 0),
                                stop=(jj == 1 and sl == NSL - 1))

            def stage_C(sp):
                st = state.pop(sp)
                g0 = sp * EW
                acc = st["acc"]
                acc_v = acc[:, EW:2 * EW]
                wgt_sc, wgt_v = st["wgt_sc"], st["wgt_v"]
                sg_sc = postp.tile([S, EW], bf16, tag="sg_sc", name=f"sg_sc_{sp}")
                nc.scalar.activation(sg_sc[:], acc[0:S, 0:EW], Sigm)
                sg_g = postp.tile([V, EW], bf16, tag="sg_g", name=f"sg_g_{sp}")
                nc.scalar.activation(sg_g[:], acc[S:S + V, 0:EW], Sigm)

                sgw = postp.tile([S, EW], bf16, tag="sgw", name=f"sgw_{sp}")
                nc.vector.tensor_tensor(sgw[:], sg_sc[:], wgt_sc[:], mul_op)
                gwv = postp.tile([V, EW], bf16, tag="gwv", name=f"gwv_{sp}")
                nc.vector.tensor_tensor(gwv[:], sg_g[:], wgt_v[:], mul_op)
                gwrep_p = pmlp.tile([3 * V, EW], f32, tag="mlp", name=f"gwrep_{sp}")
                nc.tensor.matmul(gwrep_p[:], sel3, gwv[:], start=True, stop=True)
                gwrep = postp.tile([3 * V, EW], bf16, tag="gwrep", name=f"gwrep_sb_{sp}")
                nc.scalar.activation(gwrep[:], gwrep_p[:], Copy)

                accv_sb = postp.tile([3 * V, EW], bf16, tag="accv_sb", name=f"accv_sb_{sp}")
                nc.scalar.activation(accv_sb[:], acc_v[0:3 * V, :], Copy)
                accs_sb = postp.tile([S, EW], bf16, tag="accs_sb", name=f"accs_sb_{sp}")
                nc.scalar.activation(accs_sb[:], acc[0:S, 0:EW], Copy)
                osc = postp.tile([S, EW], bf16, tag="osc", name=f"osc_{sp}")
                nc.gpsimd.tensor_tensor(osc[:], accs_sb[:], sgw[:], mul_op)
                ovec = postp.tile([3 * V, EW], bf16, tag="ovec", name=f"ovec_{sp}")
                nc.vector.tensor_tensor(ovec[:], accv_sb[:], gwrep[:], mul_op)

                nc.sync.dma_start(d_osc[:, g0:g0 + EW], osc[:])
                nc.sync.dma_start(d_ovec[:, g0:g0 + EW], ovec[:])

            def due(base, r):
                # emit leg for sp where max(0, sp-base) == r
                if r == 0:
                    return [sp for sp in range(0, min(base + 1, NSUP))]
                sp = r + base
                return [sp] if sp < NSUP else []

            for sp in due(2, 0):
                load_fwT(sp)
            for r in range(NSUP + 2):
                if r < NSUP:
                    loads(r)
                for sp in due(2, r + 1):
                    load_fwT(sp)
                if 1 <= r <= NSUP:
                    stage_B2(r - 1, 1)  # s1 chunk matmuls: ktrans-s1 just landed
                    stage_B1(r - 1, 0)  # mults for i=0,1 (ready at round start)
                if r < NSUP:
                    prebuilds(r)
                for sp in due(2, r):
                    mlp1(sp)
                if 1 <= r <= NSUP:
                    stage_B1(r - 1, 1)  # mults for i=2
                if r < NSUP:
                    krons(r, 0)
                    step1(r, 0)
                    krons(r, 1)
                    step1(r, 1)
                if r >= 2:
                    stage_C(r - 2)
                if 1 <= r <= NSUP:
                    stage_B3(r - 1)   # PE reduces after this round's step1
                if r < NSUP:
                    stage_B2(r, 0)    # s0 chunk matmuls late this round
                for sp in due(1, r):
                    mlp1g(sp)
                    mlp2(sp)
                for sp in due(0, r):
                    mlp2g(sp)
                    mlp3(sp)

    nc.compile()
    return nc


_CACHED = {}


def kernel(fea_in1, fea_in2, fea_weight,
           w_ss_s, w_vv_s, w_ss_g, w_vv_g, w_sv_v, w_vs_v,
           fc_w1, fc_b1, fc_w2, fc_b2, fc_w3, fc_b3, batch_edge):
    fea_in1 = np.asarray(fea_in1, dtype=np.float32)
    fea_in2 = np.asarray(fea_in2, dtype=np.float32)
    fea_weight = np.asarray(fea_weight, dtype=np.float32)

    wd = _prep_weights(np.asarray(w_ss_s, np.float32), np.asarray(w_vv_s, np.float32),
                       np.asarray(w_ss_g, np.float32), np.asarray(w_vv_g, np.float32),
                       np.asarray(w_sv_v, np.float32), np.asarray(w_vs_v, np.float32),
                       np.asarray(fc_w1, np.float32), np.asarray(fc_b1, np.float32),
                       np.asarray(fc_w2, np.float32), np.asarray(fc_b2, np.float32),
                       np.asarray(fc_w3, np.float32), np.asarray(fc_b3, np.float32))

    if "nc" not in _CACHED:
        _CACHED["nc"] = _build_program()
    nc = _CACHED["nc"]

    in_maps = []
    for c in range(NCORES):
        s0 = c * EC
        f1 = np.zeros((EPAD, 160), BF16)
        f1[:EC] = fea_in1[s0:s0 + EC].astype(BF16)
        f2 = np.zeros((EPAD, 160), BF16)
        f2[:EC] = fea_in2[s0:s0 + EC].astype(BF16)
        fea = np.concatenate([f1, f2], axis=1)  # [EPAD, 320]

        feaT = np.zeros((NSUP, S, 2, EW), BF16)
        feaT[:, :, 0, :] = f1[:, :S].reshape(NSUP, EW, S).transpose(0, 2, 1)
        feaT[:, :, 1, :] = f2[:, :S].reshape(NSUP, EW, S).transpose(0, 2, 1)

        # stacks: [NSUP, p=(dw,v), j, e]; j=2i -> x2v[e, v=p%32, i]; j=2i+1 -> x1v
        x1v = f1[:, S:].reshape(EPAD, V, 3)
        x2v = f2[:, S:].reshape(EPAD, V, 3)
        stk = np.empty((NSUP, 128, 6, EW), BF16)
        for i in range(3):
            s2 = x2v[:, :, i].T.reshape(1, V, NSUP, EW)
            s1 = x1v[:, :, i].T.reshape(1, V, NSUP, EW)
            stk[:, :, 2 * i, :] = np.broadcast_to(s2, (4, V, NSUP, EW)) \
                .reshape(128, NSUP, EW).transpose(1, 0, 2)
            stk[:, :, 2 * i + 1, :] = np.broadcast_to(s1, (4, V, NSUP, EW)) \
                .reshape(128, NSUP, EW).transpose(1, 0, 2)

        fwT = np.zeros((FC_IN, EPAD), BF16)
        fwT[:, :EC] = fea_weight[s0:s0 + EC].T.astype(BF16)
        m = {"fea": fea, "feaT": feaT, "stk": stk, "fwT": fwT}
        m.update(wd)
        in_maps.append(m)

    import os
    trace = bool(int(os.environ.get("KERNEL_TRACE", "0")))
    res = run_bass_kernel_spmd(nc, in_maps, core_ids=list(range(NCORES)), trace=trace)
    _CACHED["exec_time_ns"] = res.exec_time_ns

    out = np.empty((E, S + 3 * V), np.float32)
    # vec partition p = i*32+w  ->  output column 64 + 3*w + i
    vec_cols = np.empty(3 * V, np.int64)
    for i in range(3):
        for w in range(V):
            vec_cols[i * V + w] = S + 3 * w + i
    for c in range(NCORES):
        s0 = c * EC
        osc = np.asarray(res.results[c]["out_sc"], dtype=np.float32)[:, :EC]
        ovec = np.asarray(res.results[c]["out_vec"], dtype=np.float32)[:, :EC]
        out[s0:s0 + EC, :S] = osc.T
        out[s0:s0 + EC, vec_cols] = ovec.T
    return out


if __name__ == "__main__":
    rng = np.random.default_rng(0)
    ins = {
        "fea_in1": rng.standard_normal((E, 160)).astype(np.float32),
        "fea_in2": rng.standard_normal((E, 160)).astype(np.float32),
        "fea_weight": rng.standard_normal((E, FC_IN)).astype(np.float32),
        "w_ss_s": rng.standard_normal((S, S, S)).astype(np.float32),
        "w_vv_s": rng.standard_normal((V, V, S)).astype(np.float32),
        "w_ss_g": rng.standard_normal((S, S, V)).astype(np.float32),
        "w_vv_g": rng.standard_normal((V, V, V)).astype(np.float32),
        "w_sv_v": rng.standard_normal((S, V, V)).astype(np.float32),
        "w_vs_v": rng.standard_normal((V, S, V)).astype(np.float32),
        "fc_w1": rng.standard_normal((FC_IN, HID)).astype(np.float32),
        "fc_b1": np.zeros(HID, np.float32),
        "fc_w2": rng.standard_normal((HID, HID)).astype(np.float32),
        "fc_b2": np.zeros(HID, np.float32),
        "fc_w3": rng.standard_normal((HID, S + V)).astype(np.float32),
        "fc_b3": np.zeros(S + V, np.float32),
        "batch_edge": np.zeros(E, np.int32),
    }
    out = kernel(**ins)
    print("kernel out", out.shape, out.dtype, float(np.abs(out).mean()))


# revision 28
# speedup vs baseline: 1.0372x; 1.0077x over previous
"""Trainium2 Bass kernel for nn_EquiConv (e3nn-style FullyConnectedTensorProduct
+ gate + radial-MLP elementwise conv), data-parallel over edges on 8 cores.

v4 architecture (per core, 256-edge supertiles of two 128-edge subtiles):
  - ss/vv paths: DVE/GPSIMD build per-edge outer-product krons edge-major
    (per subtile), one DMA-transpose per subtile flips all 40 k-chunks into a
    shared k-major SBUF supertile, PE runs 40 chunk matmuls at 256-wide.
  - sv/vs paths: factorized. PE contracts the 64-wide scalar side first
    (weights stationary, x1sT/x2sT moving, per subtile into PSUM), ACT evicts
    to a shared bf16 supertile, DVE/GPSIMD multiply by host-replicated
    x2v/x1v "stack" operands (one instr per path-i), PE selector-matmuls
    reduce over v and scatter slice rows into the vec PSUM feature-major.
  - MLP (3 matmuls + Silu on ACT) at 256 wide; gate + elementwise conv fused
    feature-major; bf16 outputs, host reassembles/transposes.
"""

import sys

sys.path.insert(0, "/opt/trn_rl_repo")

import numpy as np
import ml_dtypes

import concourse.bass as bass
import concourse.bacc as bacc
import concourse.mybir as mybir
import concourse.tile as tile
from concourse.bass_utils import run_bass_kernel_spmd

BF16 = ml_dtypes.bfloat16

E = 20000
S = 64
V = 32
FC_IN = 128
HID = 64
INV_SQRT3 = 0.5773502691896258

NCORES = 8
EC = E // NCORES  # 2500 edges per core
ET = 128  # edges per subtile
NT = (EC + ET - 1) // ET  # 20 subtiles
EPAD = NT * ET  # 2560
NSUB = 2
EW = NSUB * ET  # 256 edges per supertile
NSUP = EPAD // EW  # 10

A_SC = float(1.0 / np.sqrt(np.float32(S * S + V * V)))
A_VEC = float(1.0 / np.sqrt(np.float32(2 * S * V)))

f32 = mybir.dt.float32
bf16 = mybir.dt.bfloat16

N_SS = (S * S) // 128  # 32 ss chunks
N_VV = (V * V) // 128  # 8 vv chunks (i-summed)
NCH = N_SS + N_VV      # 40 chunks -> 96-wide out (sc|g)
NSL = 8                # (dw,v) slices per sv/vs step-1 (8 x 128 rows)

# packed-constant column offsets (bf16 [128, WCONST])
OFF_WSSVV = 0
OFF_WSV = OFF_WSSVV + NCH * (S + V)
OFF_WVS = OFF_WSV + NSL * 4 * V
OFF_SEL = OFF_WVS + NSL * 4 * V
OFF_FC1 = OFF_SEL + NSL * V
OFF_FC2 = OFF_FC1 + HID
OFF_FC3 = OFF_FC2 + HID
OFF_SEL3 = OFF_FC3 + S + V
WCONST = OFF_SEL3 + 3 * V


def _prep_weights(w_ss_s, w_vv_s, w_ss_g, w_vv_g, w_sv_v, w_vs_v,
                  fc_w1, fc_b1, fc_w2, fc_b2, fc_w3, fc_b3):
    """Host-side rearrangement of the shared weights."""
    wss = np.concatenate([w_ss_s, w_ss_g], axis=2) * A_SC  # [64,64,96]
    wvv = np.concatenate([w_vv_s, w_vv_g], axis=2) * (A_SC * INV_SQRT3)  # [32,32,96]
    w_ssvv = np.concatenate(
        [wss.reshape(S * S, S + V), wvv.reshape(V * V, S + V)], axis=0
    )  # [5120, 96];  k = u*64+v (ss) ++ 4096 + u*32+v (vv)
    w_ssvv = (
        w_ssvv.reshape(NCH, 128, S + V).transpose(1, 0, 2)
        .reshape(128, NCH * (S + V))
    )

    # sv step1 stationary: [u, (s,dw,v)] = w_sv_v[u, v, s*4+dw] * A_VEC
    wsv_mat = (w_sv_v * A_VEC).transpose(0, 2, 1).reshape(S, NSL * 4 * V)
    # vs step1 stationary: [vs, (s,dw,uv)] = w_vs_v[uv, vs, s*4+dw] * A_VEC
    wvs_mat = (w_vs_v * A_VEC).transpose(1, 2, 0).reshape(S, NSL * 4 * V)

    # selector for the v-reduce: sel[p=(dw,v), s, w'] = 1 iff w' == s*4 + p//32
    sel = np.zeros((128, NSL, V), dtype=np.float32)
    for p in range(128):
        dw = p // 32
        for s in range(NSL):
            sel[p, s, s * 4 + dw] = 1.0

    sel3 = np.zeros((V, 3 * V), dtype=np.float32)  # replicate [32] -> [(i,w)=96]
    for i in range(3):
        for w in range(V):
            sel3[w, i * V + w] = 1.0

    wpack = np.zeros((128, WCONST), BF16)
    wpack[:, OFF_WSSVV:OFF_WSSVV + NCH * (S + V)] = w_ssvv.astype(BF16)
    wpack[0:S, OFF_WSV:OFF_WSV + NSL * 4 * V] = wsv_mat.astype(BF16)
    wpack[0:S, OFF_WVS:OFF_WVS + NSL * 4 * V] = wvs_mat.astype(BF16)
    wpack[:, OFF_SEL:OFF_SEL + NSL * V] = sel.reshape(128, NSL * V).astype(BF16)
    wpack[0:FC_IN, OFF_FC1:OFF_FC1 + HID] = fc_w1.astype(BF16)
    wpack[0:HID, OFF_FC2:OFF_FC2 + HID] = fc_w2.astype(BF16)
    wpack[0:HID, OFF_FC3:OFF_FC3 + S + V] = fc_w3.astype(BF16)
    wpack[0:V, OFF_SEL3:OFF_SEL3 + 3 * V] = sel3.astype(BF16)
    bpack = np.zeros((S + V, 3), np.float32)
    bpack[0:HID, 0] = fc_b1
    bpack[0:HID, 1] = fc_b2
    bpack[:, 2] = fc_b3
    return {"wpack": wpack, "bpack": bpack}


def _build_program():
    nc = bacc.Bacc("TRN2", target_bir_lowering=False, debug=False)

    d_fea = nc.dram_tensor("fea", [EPAD, 320], bf16, kind="ExternalInput").ap()
    d_feaT = nc.dram_tensor("feaT", [NSUP, S, 2, EW], bf16, kind="ExternalInput").ap()
    d_stk = nc.dram_tensor("stk", [NSUP, 128, 6, EW], bf16, kind="ExternalInput").ap()
    d_fwT = nc.dram_tensor("fwT", [FC_IN, EPAD], bf16, kind="ExternalInput").ap()
    d_wpack = nc.dram_tensor("wpack", [128, WCONST], bf16, kind="ExternalInput").ap()
    d_bpack = nc.dram_tensor("bpack", [S + V, 3], f32, kind="ExternalInput").ap()

    d_osc = nc.dram_tensor("out_sc", [S, EPAD], bf16, kind="ExternalOutput").ap()
    d_ovec = nc.dram_tensor("out_vec", [3 * V, EPAD], bf16, kind="ExternalOutput").ap()

    SiLU = mybir.ActivationFunctionType.Silu
    Sigm = mybir.ActivationFunctionType.Sigmoid
    Copy = mybir.ActivationFunctionType.Copy
    Ident = mybir.ActivationFunctionType.Identity
    mul_op = mybir.AluOpType.mult
    add_op = mybir.AluOpType.add

    with tile.TileContext(nc) as tc:
        with (
            tc.tile_pool(name="consts", bufs=1) as consts,
            tc.tile_pool(name="io", bufs=3) as io,
            tc.tile_pool(name="kron", bufs=2) as kronp,
            tc.tile_pool(name="ktr", bufs=2) as ktrp,
            tc.tile_pool(name="tsb", bufs=2) as tsbp,
            tc.tile_pool(name="mm", bufs=2) as mmp,
            tc.tile_pool(name="post", bufs=4) as postp,
            tc.tile_pool(name="pT", bufs=3, space=bass.MemorySpace.PSUM) as pT,
            tc.tile_pool(name="pacc", bufs=3, space=bass.MemorySpace.PSUM) as pacc,
            tc.tile_pool(name="pmlp", bufs=2, space=bass.MemorySpace.PSUM) as pmlp,
        ):
            # ---- constants (resident, one packed bf16 DMA + one f32 DMA) ----
            wpack = consts.tile([128, WCONST], bf16, name="wpack")
            nc.sync.dma_start(wpack[:], d_wpack)
            wssvv = wpack[:, OFF_WSSVV:OFF_WSSVV + NCH * (S + V)]
            wsv = wpack[0:S, OFF_WSV:OFF_WSV + NSL * 4 * V]
            wvs = wpack[0:S, OFF_WVS:OFF_WVS + NSL * 4 * V]
            sel = wpack[:, OFF_SEL:OFF_SEL + NSL * V]
            wfc1 = wpack[0:FC_IN, OFF_FC1:OFF_FC1 + HID]
            wfc2 = wpack[0:HID, OFF_FC2:OFF_FC2 + HID]
            wfc3 = wpack[0:HID, OFF_FC3:OFF_FC3 + S + V]
            sel3 = wpack[0:V, OFF_SEL3:OFF_SEL3 + 3 * V]
            bpack = consts.tile([S + V, 3], f32, name="bpack")
            nc.sync.dma_start(bpack[:], d_bpack)
            bfc1 = bpack[0:HID, 0:1]
            bfc2 = bpack[0:HID, 1:2]
            bfc3 = bpack[0:S + V, 2:3]

            state = {}

            def load_fwT(sp):
                fwT = io.tile([FC_IN, EW], bf16, tag="fwT", name=f"fwT_{sp}")
                nc.sync.dma_start(fwT[:], d_fwT[:, sp * EW:sp * EW + EW])
                state.setdefault(sp, {})["fwT"] = fwT

            def loads(sp):
                st = state.setdefault(sp, {})
                g0 = sp * EW
                feaT = io.tile([S, 2, EW], bf16, tag="feaT", name=f"feaT_{sp}")
                nc.sync.dma_start(feaT[:], d_feaT[sp])
                stk = io.tile([128, 6, EW], bf16, tag="stk", name=f"stk_{sp}")
                nc.sync.dma_start(stk[:], d_stk[sp])
                st["feaT"], st["stk"] = feaT, stk
                st["fea"] = []
                for s in range(NSUB):
                    e0 = g0 + s * ET
                    fea = io.tile([ET, 320], bf16, tag=f"fea_{s}", name=f"fea_{sp}_{s}")
                    nc.sync.dma_start(fea[:], d_fea[e0:e0 + ET, :])
                    st["fea"].append(fea)

            def mlp1(sp):
                st = state[sp]
                h1p = pmlp.tile([S + V, EW], f32, tag="mlp", name=f"h1p_{sp}")
                nc.tensor.matmul(h1p[0:HID, :], wfc1, st["fwT"][:], start=True, stop=True)
                h1b = postp.tile([HID, EW], bf16, tag="h1b", name=f"h1b_{sp}")
                nc.scalar.activation(h1b[:], h1p[0:HID, :], Ident, bias=bfc1)
                h1g = postp.tile([HID, EW], bf16, tag="h1g", name=f"h1g_{sp}")
                nc.scalar.activation(h1g[:], h1p[0:HID, :], Sigm, bias=bfc1)
                st["h1parts"] = (h1b, h1g)

            def mlp2(sp):
                st = state[sp]
                h2p = pmlp.tile([S + V, EW], f32, tag="mlp", name=f"h2p_{sp}")
                nc.tensor.matmul(h2p[0:HID, :], wfc2, st["h1"][:], start=True, stop=True)
                h2b = postp.tile([HID, EW], bf16, tag="h2b", name=f"h2b_{sp}")
                nc.scalar.activation(h2b[:], h2p[0:HID, :], Ident, bias=bfc2)
                h2g = postp.tile([HID, EW], bf16, tag="h2g", name=f"h2g_{sp}")
                nc.scalar.activation(h2g[:], h2p[0:HID, :], Sigm, bias=bfc2)
                st["h2parts"] = (h2b, h2g)

            def mlp1g(sp):
                st = state[sp]
                h1b, h1g = st["h1parts"]
                h1 = postp.tile([HID, EW], bf16, tag="h1", name=f"h1_{sp}")
                nc.gpsimd.tensor_tensor(h1[:], h1b[:], h1g[:], mul_op)
                st["h1"] = h1

            def mlp2g(sp):
                st = state[sp]
                h2b, h2g = st["h2parts"]
                h2 = postp.tile([HID, EW], bf16, tag="h2", name=f"h2_{sp}")
                nc.gpsimd.tensor_tensor(h2[:], h2b[:], h2g[:], mul_op)
                st["h2"] = h2

            def mlp3(sp):
                st = state[sp]
                wp = pmlp.tile([S + V, EW], f32, tag="mlp", name=f"wp_{sp}")
                nc.tensor.matmul(wp[:], wfc3, st["h2"][:], start=True, stop=True)
                wgt_sc = postp.tile([S, EW], bf16, tag="wgt_sc", name=f"wgt_sc_{sp}")
                nc.scalar.activation(wgt_sc[:], wp[0:S, :], Ident, bias=bfc3[0:S, :])
                wgt_v = postp.tile([V, EW], bf16, tag="wgt_v", name=f"wgt_v_{sp}")
                nc.scalar.activation(wgt_v[:], wp[S:S + V, :], Ident, bias=bfc3[S:S + V, :])
                st["wgt_sc"], st["wgt_v"] = wgt_sc, wgt_v

            def prebuilds(sp):
                st = state[sp]
                st["pre"] = []
                for s in range(NSUB):
                    fea = st["fea"][s]
                    x1s2 = io.tile([ET, 2 * S], bf16, tag=f"x1s2_{s}", name=f"x1s2_{sp}_{s}")
                    nc.scalar.activation(
                        x1s2[:].rearrange("e (u p) -> e u p", p=2),
                        fea[:, 0:S].unsqueeze(2).broadcast_to([ET, S, 2]), Copy)
                    x2vg = io.tile([ET, 3 * V], bf16, tag=f"x2vg_{s}", name=f"x2vg_{sp}_{s}")
                    nc.scalar.activation(
                        x2vg[:].rearrange("e (i u) -> e i u", u=V),
                        fea[:, 160 + S:320].rearrange("e (u i) -> e i u", i=3), Copy)
                    x1vg2 = io.tile([ET, 6 * V], bf16, tag=f"x1vg2_{s}", name=f"x1vg2_{sp}_{s}")
                    nc.scalar.activation(
                        x1vg2[:].rearrange("e (i u p) -> e i u p", u=V, p=2),
                        fea[:, S:160].rearrange("e (u i) -> e i u", i=3)
                            .unsqueeze(3).broadcast_to([ET, 3, V, 2]), Copy)
                    st["pre"].append((x1s2, x2vg, x1vg2))

            def krons(sp, s):
                st = state[sp]
                fea = st["fea"][s]
                x1s2, x2vg, x1vg2 = st["pre"][s]
                x2s = fea[:, 160:160 + S]
                if s == 0:
                    st["st_k"] = ktrp.tile([128, NCH, EW], bf16, tag="st_k", name=f"st_k_{sp}")
                kron = kronp.tile([ET, 5120], bf16, tag="kron", name=f"kron_{sp}_{s}")
                nc.vector.tensor_tensor(
                    kron[:, 0:S * S].rearrange("e (u vh p) -> e u vh p", vh=S // 2, p=2),
                    x1s2[:].rearrange("e (u p) -> e u p", p=2)
                        .unsqueeze(2).broadcast_to([ET, S, S // 2, 2]),
                    x2s.rearrange("e (vh p) -> e vh p", p=2)
                        .unsqueeze(1).broadcast_to([ET, S, S // 2, 2]),
                    mul_op)
                pv = [kronp.tile([ET, V * V], bf16, tag=f"pv{i}", name=f"pv{i}_{sp}_{s}") for i in range(2)]
                for i in range(3):
                    dst = (kron[:, S * S:S * S + V * V] if i == 2 else pv[i][:])
                    eng = nc.vector
                    eng.tensor_tensor(
                        dst.rearrange("e (u vh p) -> e u vh p", vh=V // 2, p=2),
                        x1vg2[:, i * 2 * V:(i + 1) * 2 * V]
                            .rearrange("e (u p) -> e u p", p=2)
                            .unsqueeze(2).broadcast_to([ET, V, V // 2, 2]),
                        x2vg[:, i * V:(i + 1) * V]
                            .rearrange("e (vh p) -> e vh p", p=2)
                            .unsqueeze(1).broadcast_to([ET, V, V // 2, 2]),
                        mul_op)
                kvv = kron[:, S * S:S * S + V * V]
                nc.vector.tensor_tensor(kvv, kvv, pv[0][:], add_op)
                nc.vector.tensor_tensor(kvv, kvv, pv[1][:], add_op)
                nc.sync.dma_start(st["st_k"][:, :, s * ET:(s + 1) * ET], kron[:],
                                  transpose=True)

            def step1(sp, s):
                st = state[sp]
                feaT = st["feaT"]
                if s == 0:
                    st["Tsv"] = tsbp.tile([128, NSL, EW], bf16, tag="Tsv", name=f"Tsv_sb_{sp}")
                    st["Tvs"] = tsbp.tile([128, NSL, EW], bf16, tag="Tvs", name=f"Tvs_sb_{sp}")
                Tsv, Tvs = st["Tsv"], st["Tvs"]
                NH = NSL // 2
                for path, (wmat, T) in enumerate(((wsv, Tsv), (wvs, Tvs))):
                    for h in range(2):
                        Tp = pT.tile([128, NH * ET], f32, tag="T", name=f"T_{sp}_{s}_{path}_{h}")
                        for sl in range(NH):
                            gsl = h * NH + sl
                            nc.tensor.matmul(
                                Tp[:, sl * ET:(sl + 1) * ET],
                                wmat[:, gsl * 128:(gsl + 1) * 128],
                                feaT[:, path, s * ET:(s + 1) * ET],
                                start=True, stop=True)
                        nc.scalar.activation(
                            T[:, h * NH:(h + 1) * NH, s * ET:(s + 1) * ET],
                            Tp[:].rearrange("p (sl e) -> p sl e", e=ET),
                            Copy)

            def stage_B1(sp, half):
                st = state[sp]
                stk, Tsv, Tvs = st["stk"], st["Tsv"], st["Tvs"]
                if 0 in half:
                    st["ms"] = []
                NGD = 4
                for i in half:
                    for jj, T in ((0, Tsv), (1, Tvs)):
                        j = 2 * i + jj
                        m = mmp.tile([128, NSL, EW], bf16, tag=f"m{j}", name=f"m{j}_{sp}")
                        nc.gpsimd.tensor_tensor(
                            m[:, NSL - NGD:NSL, :], T[:, NSL - NGD:NSL, :],
                            stk[:, j, :].unsqueeze(1).broadcast_to([128, NGD, EW]),
                            mul_op)
                        nc.vector.tensor_tensor(
                            m[:, 0:NSL - NGD, :], T[:, 0:NSL - NGD, :],
                            stk[:, j, :].unsqueeze(1).broadcast_to([128, NSL - NGD, EW]),
                            mul_op)
                        st["ms"].append(m)

            def stage_B2(sp, s):
                st = state[sp]
                if s == 0:
                    st["acc"] = pacc.tile([S + V, 2 * EW], f32, tag="acc", name=f"acc_{sp}")
                acc_ss = st["acc"][:, 0:EW]
                st_k = st["st_k"]
                for c in range(NCH):
                    nc.tensor.matmul(
                        acc_ss[:, s * ET:(s + 1) * ET],
                        wssvv[:, c * (S + V):(c + 1) * (S + V)],
                        st_k[:, c, s * ET:(s + 1) * ET],
                        start=(c == 0), stop=(c == NCH - 1))

            def stage_B3(sp):
                st = state[sp]
                acc_v = st["acc"][:, EW:2 * EW]
                for i in range(3):
                    for jj in range(2):
                        m = st["ms"][2 * i + jj]
                        for sl in range(NSL):
                            nc.tensor.matmul(
                                acc_v[i * V:(i + 1) * V, :],
                                sel[:, sl * V:(sl + 1) * V],
                                m[:, sl, :],
                                start=(jj == 0 and sl == 0),
                                stop=(jj == 1 and sl == NSL - 1))

            def stage_C(sp):
                st = state.pop(sp)
                g0 = sp * EW
                acc = st["acc"]
                acc_v = acc[:, EW:2 * EW]
                wgt_sc, wgt_v = st["wgt_sc"], st["wgt_v"]
                sg_sc = postp.tile([S, EW], bf16, tag="sg_sc", name=f"sg_sc_{sp}")
                nc.scalar.activation(sg_sc[:], acc[0:S, 0:EW], Sigm)
                sg_g = postp.tile([V, EW], bf16, tag="sg_g", name=f"sg_g_{sp}")
                nc.scalar.activation(sg_g[:], acc[S:S + V, 0:EW], Sigm)

                sgw = postp.tile([S, EW], bf16, tag="sgw", name=f"sgw_{sp}")
                nc.vector.tensor_tensor(sgw[:], sg_sc[:], wgt_sc[:], mul_op)
                gwv = postp.tile([V, EW], bf16, tag="gwv", name=f"gwv_{sp}")
                nc.vector.tensor_tensor(gwv[:], sg_g[:], wgt_v[:], mul_op)
                gwrep_p = pmlp.tile([3 * V, EW], f32, tag="mlp", name=f"gwrep_{sp}")
                nc.tensor.matmul(gwrep_p[:], sel3, gwv[:], start=True, stop=True)
                gwrep = postp.tile([3 * V, EW], bf16, tag="gwrep", name=f"gwrep_sb_{sp}")
                nc.scalar.activation(gwrep[:], gwrep_p[:], Copy)

                accv_sb = postp.tile([3 * V, EW], bf16, tag="accv_sb", name=f"accv_sb_{sp}")
                nc.scalar.activation(accv_sb[:], acc_v[0:3 * V, :], Copy)
                accs_sb = postp.tile([S, EW], bf16, tag="accs_sb", name=f"accs_sb_{sp}")
                nc.scalar.activation(accs_sb[:], acc[0:S, 0:EW], Copy)
                osc = postp.tile([S, EW], bf16, tag="osc", name=f"osc_{sp}")
                nc.gpsimd.tensor_tensor(osc[:], accs_sb[:], sgw[:], mul_op)
                ovec = postp.tile([3 * V, EW], bf16, tag="ovec", name=f"ovec_{sp}")
                nc.vector.tensor_tensor(ovec[:], accv_sb[:], gwrep[:], mul_op)

                nc.sync.dma_start(d_osc[:, g0:g0 + EW], osc[:])
                nc.sync.dma_start(d_ovec[:, g0:g0 + EW], ovec[:])

            def due(base, r):
                # emit leg for sp where max(0, sp-base) == r
                if r == 0:
                    return [sp for sp in range(0, min(base + 1, NSUP))]
                sp = r + base
                return [sp] if sp < NSUP else []

            for sp in due(2, 0):
                load_fwT(sp)
            for r in range(NSUP + 2):
                if r < NSUP:
                    loads(r)
                for sp in due(2, r + 1):
                    load_fwT(sp)
                if 1 <= r <= NSUP:
                    stage_B2(r - 1, 1)  # s1 chunk matmuls: ktrans-s1 just landed
                    stage_B1(r - 1, (0, 1))  # mults for i=0,1 (ready at round start)
                if r < NSUP:
                    prebuilds(r)
                for sp in due(2, r):
                    mlp1(sp)
                if 1 <= r <= NSUP:
                    stage_B1(r - 1, (2,))  # mults for i=2
                if r < NSUP:
                    krons(r, 0)
                    step1(r, 0)
                    krons(r, 1)
                    step1(r, 1)
                if r >= 2:
                    stage_C(r - 2)
                if 1 <= r <= NSUP:
                    stage_B3(r - 1)   # PE reduces after this round's step1
                if r < NSUP:
                    stage_B2(r, 0)    # s0 chunk matmuls late this round
                for sp in due(1, r):
                    mlp1g(sp)
                    mlp2(sp)
                for sp in due(0, r):
                    mlp2g(sp)
                    mlp3(sp)

    nc.compile()
    return nc


_CACHED = {}


def kernel(fea_in1, fea_in2, fea_weight,
           w_ss_s, w_vv_s, w_ss_g, w_vv_g, w_sv_v, w_vs_v,
           fc_w1, fc_b1, fc_w2, fc_b2, fc_w3, fc_b3, batch_edge):
    fea_in1 = np.asarray(fea_in1, dtype=np.float32)
    fea_in2 = np.asarray(fea_in2, dtype=np.float32)
    fea_weight = np.asarray(fea_weight, dtype=np.float32)

    wd = _prep_weights(np.asarray(w_ss_s, np.float32), np.asarray(w_vv_s, np.float32),
                       np.asarray(w_ss_g, np.float32), np.asarray(w_vv_g, np.float32),
                       np.asarray(w_sv_v, np.float32), np.asarray(w_vs_v, np.float32),
                       np.asarray(fc_w1, np.float32), np.asarray(fc_b1, np.float32),
                       np.asarray(fc_w2, np.float32), np.asarray(fc_b2, np.float32),
                       np.asarray(fc_w3, np.float32), np.asarray(fc_b3, np.float32))

    if "nc" not in _CACHED:
        _CACHED["nc"] = _build_program()
    nc = _CACHED["nc"]

    in_maps = []
    for c in range(NCORES):
        s0 = c * EC
        f1 = np.zeros((EPAD, 160), BF16)
        f1[:EC] = fea_in1[s0:s0 + EC].astype(BF16)
        f2 = np.zeros((EPAD, 160), BF16)
        f2[:EC] = fea_in2[s0:s0 + EC].astype(BF16)
        fea = np.concatenate([f1, f2], axis=1)  # [EPAD, 320]

        feaT = np.zeros((NSUP, S, 2, EW), BF16)
        feaT[:, :, 0, :] = f1[:, :S].reshape(NSUP, EW, S).transpose(0, 2, 1)
        feaT[:, :, 1, :] = f2[:, :S].reshape(NSUP, EW, S).transpose(0, 2, 1)

        # stacks: [NSUP, p=(dw,v), j, e]; j=2i -> x2v[e, v=p%32, i]; j=2i+1 -> x1v
        x1v = f1[:, S:].reshape(EPAD, V, 3)
        x2v = f2[:, S:].reshape(EPAD, V, 3)
        stk = np.empty((NSUP, 128, 6, EW), BF16)
        for i in range(3):
            s2 = x2v[:, :, i].T.reshape(1, V, NSUP, EW)
            s1 = x1v[:, :, i].T.reshape(1, V, NSUP, EW)
            stk[:, :, 2 * i, :] = np.broadcast_to(s2, (4, V, NSUP, EW)) \
                .reshape(128, NSUP, EW).transpose(1, 0, 2)
            stk[:, :, 2 * i + 1, :] = np.broadcast_to(s1, (4, V, NSUP, EW)) \
                .reshape(128, NSUP, EW).transpose(1, 0, 2)

        fwT = np.zeros((FC_IN, EPAD), BF16)
        fwT[:, :EC] = fea_weight[s0:s0 + EC].T.astype(BF16)
        m = {"fea": fea, "feaT": feaT, "stk": stk, "fwT": fwT}
        m.update(wd)
        in_maps.append(m)

    import os
    trace = bool(int(os.environ.get("KERNEL_TRACE", "0")))
    res = run_bass_kernel_spmd(nc, in_maps, core_ids=list(range(NCORES)), trace=trace)
    _CACHED["exec_time_ns"] = res.exec_time_ns

    out = np.empty((E, S + 3 * V), np.float32)
    # vec partition p = i*32+w  ->  output column 64 + 3*w + i
    vec_cols = np.empty(3 * V, np.int64)
    for i in range(3):
        for w in range(V):
            vec_cols[i * V + w] = S + 3 * w + i
    for c in range(NCORES):
        s0 = c * EC
        osc = np.asarray(res.results[c]["out_sc"], dtype=np.float32)[:, :EC]
        ovec = np.asarray(res.results[c]["out_vec"], dtype=np.float32)[:, :EC]
        out[s0:s0 + EC, :S] = osc.T
        out[s0:s0 + EC, vec_cols] = ovec.T
    return out


if __name__ == "__main__":
    rng = np.random.default_rng(0)
    ins = {
        "fea_in1": rng.standard_normal((E, 160)).astype(np.float32),
        "fea_in2": rng.standard_normal((E, 160)).astype(np.float32),
        "fea_weight": rng.standard_normal((E, FC_IN)).astype(np.float32),
        "w_ss_s": rng.standard_normal((S, S, S)).astype(np.float32),
        "w_vv_s": rng.standard_normal((V, V, S)).astype(np.float32),
        "w_ss_g": rng.standard_normal((S, S, V)).astype(np.float32),
        "w_vv_g": rng.standard_normal((V, V, V)).astype(np.float32),
        "w_sv_v": rng.standard_normal((S, V, V)).astype(np.float32),
        "w_vs_v": rng.standard_normal((V, S, V)).astype(np.float32),
        "fc_w1": rng.standard_normal((FC_IN, HID)).astype(np.float32),
        "fc_b1": np.zeros(HID, np.float32),
        "fc_w2": rng.standard_normal((HID, HID)).astype(np.float32),
        "fc_b2": np.zeros(HID, np.float32),
        "fc_w3": rng.standard_normal((HID, S + V)).astype(np.float32),
        "fc_b3": np.zeros(S + V, np.float32),
        "batch_edge": np.zeros(E, np.int32),
    }
    out = kernel(**ins)
    print("kernel out", out.shape, out.dtype, float(np.abs(out).mean()))
